# revision 28
# baseline (speedup 1.0000x reference)
"""Trainium2 Bass kernel for DecoderAttentionRotary.

Problem: B=1, L=4096, D=1024, H=16 heads of d=64.
  qkv = x @ Wqkv + b; q,k get rotary embedding; causal attention per head.

Sharding: tensor parallel over heads — 8 cores x 2 heads each. Each core gets
the full (host-pre-transposed) activations plus its own column shard of Wqkv,
computes its 2 heads' attention output [L, 128] and the host concatenates.

Device-side layout choices:
  - x is fed pre-transposed (xT [D, L]) so the QKV projection produces
    q^T/k^T/v^T [128, L] directly (contraction dim on partitions).
  - Scores are computed transposed (S^T = K @ Q^T) so softmax probs come out
    in [k, q] layout, which is exactly the lhsT-free layout PV needs
    (out^T = [V|1]^T @ P^T accumulated over k blocks; the |1 column yields the
    softmax denominator for free).
  - RoPE pairs are laid out 16 partitions apart within 32-partition quadrants
    (via a host-side permutation of Wq/Wk columns) so the pair swap is a
    single DVE stream_shuffle.
  - matmuls run with float32r operand views: full PE rate at N>=256 while
    keeping ~fp32 precision.
"""

import sys

for _p in ("/opt/trn_rl_repo",):
    if _p not in sys.path:
        sys.path.insert(0, _p)

import numpy as np

import concourse.bass as bass
import concourse.mybir as mybir
import concourse.tile as tile
from concourse import bacc
from concourse import bass_utils
from concourse.masks import make_identity

F32 = mybir.dt.float32
F32R = mybir.dt.float32r
AFT = mybir.ActivationFunctionType

N_CORES = 8
NUM_HEADS = 16
HPC = NUM_HEADS // N_CORES  # heads per core = 2


class Cfg:
    def __init__(self, L=4096, D=1024, d=64, CH=3, proj_copy="act",
                 tables_swdge=False, probe=""):
        self.proj_copy = proj_copy
        self.tables_swdge = tables_swdge
        self.probe = probe
        self.L = L          # sequence length
        self.D = D          # model dim
        self.d = d          # head dim
        self.P = 128
        self.LB = 512       # projection l-block
        self.KB = 128       # key block
        self.QB = 512       # query block
        self.CH = CH        # k-blocks per exp chunk
        self.NLB = L // self.LB
        self.NKB = L // self.KB
        self.NQB = L // self.QB
        self.DK = D // self.P  # contraction tiles for projection


# Permutation of head-dim components: partition p (within a head's 64 rows)
# holds component comp(p).  Pairs (2i, 2i+1) end up 16 partitions apart inside
# one 32-partition quadrant, so stream_shuffle([16..31,0..15]) swaps pairs.
def _head_perm():
    perm = np.zeros(64, dtype=np.int64)
    for p in range(64):
        g, r = p // 32, p % 32
        perm[p] = 2 * (16 * g + (r % 16)) + (1 if r >= 16 else 0)
    return perm


_PERM = _head_perm()
_SWAP_MASK = [(i + 16) % 32 for i in range(32)]
_MASK_NEG = -1.0e30


def _build_program(cfg: Cfg, nrep: int = 1):
    """Build (and bacc-compile) the per-core SPMD program.

    nrep>1 wraps the whole body in a hardware For_i loop (benchmark mode:
    one dispatch runs the kernel nrep times so device time is measurable
    above the axon dispatch floor)."""
    P, L, d = cfg.P, cfg.L, cfg.d
    nc = bacc.Bacc(
        "TRN2",
        target_bir_lowering=False,
        debug=False,
        enable_asserts=False,
        num_devices=N_CORES,
    )

    xT_d = nc.dram_tensor("xT", [cfg.D, L], F32R, kind="ExternalInput")
    w_d = nc.dram_tensor("w", [cfg.D, 3 * HPC * d], F32R, kind="ExternalInput")
    b_d = nc.dram_tensor("b", [HPC * d, 3], F32, kind="ExternalInput")
    ropec_d = nc.dram_tensor("ropeC", [P, L], F32, kind="ExternalInput")
    ropes_d = nc.dram_tensor("ropeS", [P, L], F32, kind="ExternalInput")
    mask_d = nc.dram_tensor("mask", [P, P], F32, kind="ExternalInput")
    y_d = nc.dram_tensor("y", [HPC, d, L], F32, kind="ExternalOutput")

    scale = 1.0 / float(np.sqrt(d))

    import contextlib

    with tile.TileContext(nc) as tc:
        rep_ctx = tc.For_i(0, nrep, 1) if nrep > 1 else contextlib.nullcontext()
        with (
            rep_ctx,
            tc.tile_pool(name="const", bufs=1) as const,
            tc.tile_pool(name="pers", bufs=1) as pers,
        ):
            ident = const.tile([P, P], F32, name="ident")
            make_identity(nc, ident)
            tdma = nc.gpsimd if cfg.tables_swdge else nc.sync
            mask_sb = const.tile([P, P], F32, name="mask_sb")
            tdma.dma_start(mask_sb[:], mask_d.ap())
            b_sb = const.tile([HPC * d, 3], F32, name="b_sb")
            tdma.dma_start(b_sb[:], b_d.ap())
            w_sb = const.tile([P, cfg.DK, 3 * HPC * d], F32R, name="w_sb")
            nc.sync.dma_start(w_sb[:], w_d.ap().rearrange("(o p) c -> p o c", p=P))
            ropec = const.tile([P, L], F32, name="ropec")
            tdma.dma_start(ropec[:], ropec_d.ap())
            ropes = const.tile([P, L], F32, name="ropes")
            tdma.dma_start(ropes[:], ropes_d.ap())

            ones_f = const.tile([P, 1], F32, name="ones_f")
            nc.vector.memset(ones_f[:], 1.0)
            ones_r = const.tile([P, 1], F32R, name="ones_r")
            nc.vector.tensor_copy(ones_r[:], ones_f[:])
            zero_f = const.tile([P, 1], F32, name="zero_f")
            nc.vector.memset(zero_f[:], 0.0)

            # persistent transposed activations
            qR = pers.tile([P, L], F32R, name="qR")
            # per-head K with the other head's rows zeroed: lets QK run as a
            # uniform K=128 matmul (mixing K=64/K=128 geometries stalls PE)
            kRp = [pers.tile([P, L], F32R, name=f"kRp{hh}") for hh in range(HPC)]
            vT = pers.tile([P, L], F32, name="vT")
            # V in natural layout, with a ones column per head at col 64/65:
            # [p, kb, h, 66] ; lhsT slice for PV = vnat[:, kb, h, 0:65]
            vnat = pers.tile([P, cfg.NKB, HPC, 66], F32R, name="vnat")

            nc.vector.tensor_copy(
                vnat[:, :, :, 64:66],
                ones_r[:, None, None, :].to_broadcast((P, cfg.NKB, HPC, 2)),
            )
            pt_const = None
            if cfg.probe == "pe_only":
                pt_const = pers.tile([P, HPC, cfg.QB], F32R, name="pt_const")
                nc.vector.tensor_copy(
                    pt_const[:],
                    ones_r[:, None, :].to_broadcast((P, HPC, cfg.QB)),
                )

            nc.vector.tensor_copy(
                kRp[0][d:P, :], zero_f[d:P, 0:1].to_broadcast((P - d, L)))
            nc.vector.tensor_copy(
                kRp[1][0:d, :], zero_f[0:d, 0:1].to_broadcast((d, L)))

            # ------- phase 1: projection + RoPE + V transpose -------
            NB = cfg.QB // cfg.KB
            with (
                tc.tile_pool(name="xtp", bufs=2) as xtp,
                tc.tile_pool(name="qkt", bufs=3) as qkt,
                tc.tile_pool(name="projp", bufs=2, space="PSUM") as pp,
                tc.tile_pool(name="vtp", bufs=2, space="PSUM") as tpp,
            ):
                for lb in range(cfg.NLB):
                    ls = slice(lb * cfg.LB, (lb + 1) * cfg.LB)
                    xts = []
                    if cfg.probe == "attn_only":
                        xts = None
                    for dk in range(cfg.DK if xts is not None else 0):
                        xt = xtp.tile([P, cfg.LB], F32R, name=f"xt{dk}", tag=f"xt{dk}")
                        nc.sync.dma_start(xt[:], xT_d.ap()[dk * P:(dk + 1) * P, ls])
                        xts.append(xt)
                    for t, dest in (
                            () if xts is None else ((0, None), (1, None), (2, vT))):
                        ps = pp.tile([P, cfg.LB], F32, name="projps", tag="projps")
                        for dk in range(cfg.DK):
                            nc.tensor.matmul(
                                ps[:],
                                w_sb[:, dk, t * P:(t + 1) * P],
                                xts[dk][:],
                                start=(dk == 0),
                                stop=(dk == cfg.DK - 1),
                            )
                        if t == 2:
                            nc.vector.tensor_scalar_add(
                                vT[:, ls], ps[:], b_sb[:, 2:3])
                        else:
                            raw = qkt.tile([P, cfg.LB], F32, name="qkraw", tag="qkraw")
                            nc.vector.tensor_scalar_add(
                                raw[:], ps[:], b_sb[:, t:t + 1])
                            sh = qkt.tile([P, cfg.LB], F32, name="ropesh", tag="ropesh")
                            nc.vector.stream_shuffle(sh[:], raw[:], _SWAP_MASK)
                            nc.vector.tensor_mul(sh[:], sh[:], ropes[:, ls])
                            tmp = qkt.tile([P, cfg.LB], F32, name="ropet", tag="ropet")
                            nc.vector.tensor_mul(tmp[:], raw[:], ropec[:, ls])
                            if t == 0:
                                nc.vector.tensor_add(qR[:, ls], tmp[:], sh[:])
                            else:
                                nc.vector.tensor_add(
                                    kRp[0][0:d, ls], tmp[0:d, :], sh[0:d, :])
                                nc.vector.tensor_add(
                                    kRp[1][d:P, ls], tmp[d:P, :], sh[d:P, :])
                    # v^T -> V natural for this block's 4 k-blocks
                    for kb in range(
                            lb * NB, (lb + 1) * NB if xts is not None else lb * NB):
                        psv = tpp.tile([P, P], F32, name="vtps", tag="vtps")
                        nc.tensor.transpose(
                            psv[:], vT[:, kb * P:(kb + 1) * P], ident[:])
                        nc.vector.tensor_copy(
                            vnat[:, kb, :, 0:64],
                            psv[:].rearrange("p (h c) -> p h c", c=64),
                        )

            # ------- phase 2: attention (qkp triple-buffered) -------
            with (
                tc.tile_pool(name="qkp", bufs=3, space="PSUM") as qkp,
                tc.tile_pool(name="outp", bufs=1, space="PSUM") as op,
                tc.tile_pool(name="ptp", bufs=4) as ptp,
                tc.tile_pool(name="nrm", bufs=2) as nrm,
            ):
                for qb in range(cfg.NQB):
                    if cfg.probe == "proj_only":
                        continue
                    nkb = (qb + 1) * NB
                    outs = [
                        op.tile([65, cfg.QB], F32, name=f"outT{hh}", tag=f"outT{hh}")
                        for hh in range(HPC)
                    ]

                    def _col0(kb):
                        return max(0, kb - qb * NB) * cfg.KB

                    def do_qk(kb):
                        col0 = _col0(kb)
                        qk = qkp.tile(
                            [P, HPC, cfg.QB], F32, name="qkps", tag="qkps")
                        for hh in range(HPC):
                            nc.tensor.matmul(
                                qk[:, hh, col0:cfg.QB],
                                kRp[hh][:, kb * cfg.KB:(kb + 1) * cfg.KB],
                                qR[:, qb * cfg.QB + col0:(qb + 1) * cfg.QB],
                                start=True,
                                stop=True,
                            )
                        if cfg.probe != "pe_only" and kb - qb * NB >= 0:
                            nc.vector.tensor_add(
                                qk[:, :, col0:col0 + cfg.KB],
                                qk[:, :, col0:col0 + cfg.KB],
                                mask_sb[:, None, :].to_broadcast(
                                    (P, HPC, cfg.KB)),
                            )
                        return qk

                    qk_cur = do_qk(0)
                    for kb in range(nkb):
                        col0 = _col0(kb)
                        qk_next = do_qk(kb + 1) if kb + 1 < nkb else None
                        if cfg.probe == "pe_only":
                            pt = pt_const
                        else:
                            pt = ptp.tile(
                                [P, HPC, cfg.QB], F32R, name="pt", tag="pt")
                            nc.scalar.activation(
                                pt[:, :, col0:cfg.QB], qk_cur[:, :, col0:cfg.QB],
                                AFT.Exp, scale=scale,
                            )
                        for hh in range(HPC):
                            nc.tensor.matmul(
                                outs[hh][:, col0:cfg.QB],
                                vnat[:, kb, hh, 0:65],
                                pt[:, hh, col0:cfg.QB],
                                start=(kb == 0),
                                stop=(kb == nkb - 1),
                            )
                        qk_cur = qk_next
                    # normalize in transposed layout and store [d, qb-block]
                    for hh in range(HPC):
                        rec = nrm.tile([1, cfg.QB], F32, name="rec", tag="rec")
                        nc.vector.reciprocal(rec[:], outs[hh][64:65, :])
                        recb = nrm.tile([d, cfg.QB], F32, name="recb", tag="recb")
                        nc.gpsimd.partition_broadcast(recb[:], rec[:], d)
                        yt = nrm.tile([d, cfg.QB], F32, name="yt", tag="yt")
                        nc.vector.tensor_mul(yt[:], outs[hh][0:d, :], recb[:])
                        nc.scalar.dma_start(
                            y_d.ap()[hh, :, qb * cfg.QB:(qb + 1) * cfg.QB], yt[:])

    nc.compile()
    return nc


def _host_prep(cfg: Cfg, x, freqs_cis, Wqkv, bqkv):
    """Build the 8 per-core input maps (layout prep only, no math)."""
    P, L, D, d = cfg.P, cfg.L, cfg.D, cfg.d
    x = np.asarray(x, dtype=np.float32)
    freqs_cis = np.asarray(freqs_cis, dtype=np.float32)
    Wqkv = np.asarray(Wqkv, dtype=np.float32)
    bqkv = np.asarray(bqkv, dtype=np.float32)
    NH = D // d

    xT = np.ascontiguousarray(x.reshape(L, D).T)  # [D, L]

    Wq = Wqkv[:, 0:D].reshape(D, NH, d)
    Wk = Wqkv[:, D:2 * D].reshape(D, NH, d)
    Wv = Wqkv[:, 2 * D:3 * D].reshape(D, NH, d)
    bq = bqkv[0:D].reshape(NH, d)
    bk = bqkv[D:2 * D].reshape(NH, d)
    bv = bqkv[2 * D:3 * D].reshape(NH, d)

    cos = freqs_cis[:, :, 0]  # [L, d//2]
    sin = freqs_cis[:, :, 1]
    fidx = _PERM // 2                      # [64] frequency index per partition
    sgn = np.where(_PERM % 2 == 0, -1.0, 1.0).astype(np.float32)
    C_head = np.ascontiguousarray(cos[:, fidx].T)                    # [64, L]
    S_head = np.ascontiguousarray((sin[:, fidx] * sgn[None, :]).T)   # [64, L]
    ropeC = np.ascontiguousarray(np.concatenate([C_head] * HPC, axis=0))
    ropeS = np.ascontiguousarray(np.concatenate([S_head] * HPC, axis=0))

    ii = np.arange(P)
    mask = np.where(ii[None, :] >= ii[:, None], 0.0, _MASK_NEG).astype(np.float32)

    in_maps = []
    for c in range(N_CORES):
        heads = [HPC * c + i for i in range(HPC)]
        wq = np.concatenate([Wq[:, h, :][:, _PERM] for h in heads], axis=1)
        wk = np.concatenate([Wk[:, h, :][:, _PERM] for h in heads], axis=1)
        wv = np.concatenate([Wv[:, h, :] for h in heads], axis=1)
        w_core = np.ascontiguousarray(
            np.concatenate([wq, wk, wv], axis=1))            # [D, 384]
        b_core = np.ascontiguousarray(np.stack(
            [
                np.concatenate([bq[h][_PERM] for h in heads]),
                np.concatenate([bk[h][_PERM] for h in heads]),
                np.concatenate([bv[h] for h in heads]),
            ],
            axis=1,
        ).astype(np.float32))                                # [128, 3]
        in_maps.append({
            "xT": xT,
            "w": w_core,
            "b": b_core,
            "ropeC": ropeC,
            "ropeS": ropeS,
            "mask": mask,
        })
    return in_maps


_PROG_CACHE = {}


def _get_program(cfg: Cfg, nrep: int = 1):
    key = (cfg.L, cfg.D, cfg.d, cfg.CH, nrep, cfg.proj_copy, cfg.tables_swdge,
           cfg.probe)
    if key not in _PROG_CACHE:
        _PROG_CACHE[key] = _build_program(cfg, nrep=nrep)
    return _PROG_CACHE[key]


def kernel(x, freqs_cis, Wqkv, bqkv, _trace=False):
    cfg = Cfg()
    nc = _get_program(cfg)
    in_maps = _host_prep(cfg, x, freqs_cis, Wqkv, bqkv)
    res = bass_utils.run_bass_kernel_spmd(
        nc, in_maps, core_ids=list(range(N_CORES)), trace=_trace,
    )
    out = np.empty((cfg.L, cfg.D), dtype=np.float32)
    for c in range(N_CORES):
        y = res.results[c]["y"]  # [HPC, d, L]
        for hh in range(HPC):
            h = HPC * c + hh
            out[:, h * cfg.d:(h + 1) * cfg.d] = y[hh].T
    kernel._last_results = res
    return out.reshape(1, cfg.L, cfg.D)
